# revision 29
# baseline (speedup 1.0000x reference)
"""Differential cross-attention head on 8 Trainium2 NeuronCores.

Sharding: data-parallel over batch (4) x KEY-parallel over Tk (2) = 8 cores.
Core (b, j) computes, for ALL 2048 queries of batch b, the partial attention
sums over its 1024-key half: A1/A2 = sum_{k in half} exp(s{1,2}[k,q]) v[k]
and r1/r2 = sum_{k in half} exp(s{1,2}[k,q]). The host adds the two halves
(softmax denominators and numerators are additive over keys) and normalizes:
out = A1/r1 - lam*A2/r2, then transposes back.

Why Tk- instead of Tq-sharding: with Tq-sharding both cores of a pair
duplicate the FULL k/v projections (2048 keys); with Tk-sharding they
duplicate only the q projection, which is half the flops. Net -3.5us of
TensorE work per core, and the per-group PV accumulators drain sequentially
so PSUM fits without the a2-catch-up dance on the critical tail.

Per-core math in "transposed" orientation (host transposes the output):
  - qT = Wq^T @ xT            [D, 2048]  (4 query groups of 512)
  - kT = Wk^T @ encT          [D, 1024]  (local key half, 8 chunks of 128)
  - v  = encT^T @ Wv          [1024, D]
  - s12 = [k1.q1 | k2.q2]     [128, 1024] PSUM via PE 64-row-group tiling
  - e12 = exp(s12/8)          ScalarE, PSUM->SBUF, bf16
  - A12_g += v_chunk^T @ e12  accumulated in PSUM per query group
  - r12_g: DVE chain sum of e12 + ones-matmul partition reduce

Scheduling rules learned from traces:
  - The PE pushes a briefly-blocked instruction to the BACK of its ready
    queue, so issue order must have deps resolved by dispatch time; the
    engine then self-organizes around momentary blocks.
  - All inputs ride ONE HWDGE queue in strict need order (a single queue
    saturates all 16 DMA engines; two queues round-robin unfairly).
  - The PE p-state needs ~3us of continuous work to reach 2.4GHz; throwaway
    warmup matmuls bridge the launch-to-first-data window.
  - b_k is dropped: softmax over keys is invariant to the per-query shift
    it induces. b_q applied only when nonzero.
"""

import sys
from contextlib import ExitStack

import numpy as np

_TRN_REPO = "/opt/trn_rl_repo"
if _TRN_REPO not in sys.path:
    sys.path.insert(0, _TRN_REPO)

import ml_dtypes

import concourse.bass as bass
import concourse.tile as tile
from concourse import mybir
from concourse.bass import ds, ts

F32 = mybir.dt.float32
BF16 = mybir.dt.bfloat16

E = 1024
D = 128
B = 4
TQ = 2048
TK = 2048
NCORES = 8
EC = E // 128             # 8 contraction chunks for projections
G = TQ // 512             # 4 query groups of 512 (full batch-row of queries)
TKL = TK // 2             # 1024 local keys per core
C = TKL // 128            # 8 local key chunks
NTG = TKL // 512          # 2 local key projection groups
SCALE = 0.125             # 1/sqrt(64)

NP_BF16 = ml_dtypes.bfloat16


def _build(nc: bass.Bass, with_qbias: bool, with_vbias: bool):
    xT = nc.dram_tensor("xT", [G, 128, EC, 512], BF16,
                        kind="ExternalInput").ap()
    encT = nc.dram_tensor("encT", [NTG, 128, EC, 512], BF16,
                          kind="ExternalInput").ap()
    wpack = nc.dram_tensor("wpack", [128, 3, EC, D], BF16,
                           kind="ExternalInput").ap()
    bpack = nc.dram_tensor("bpack", [128, 2], F32, kind="ExternalInput").ap()
    bv = nc.dram_tensor("bv", [D], F32, kind="ExternalInput").ap()
    pvd = nc.dram_tensor("pvd", [128, G, 1024], BF16,
                         kind="ExternalOutput").ap()
    rd = nc.dram_tensor("rd", [G, 2, 512], F32, kind="ExternalOutput").ap()

    Exp = mybir.ActivationFunctionType.Exp

    with tile.TileContext(nc) as tc, ExitStack() as ctx:
        const = ctx.enter_context(tc.tile_pool(name="const", bufs=1))
        xpool = ctx.enter_context(tc.tile_pool(name="xpool", bufs=1))
        encpool = ctx.enter_context(tc.tile_pool(name="encpool", bufs=1))
        proj = ctx.enter_context(tc.tile_pool(name="proj", bufs=1))
        epool = ctx.enter_context(tc.tile_pool(name="epool", bufs=8))
        rpool = ctx.enter_context(tc.tile_pool(name="rpool", bufs=3))
        outp = ctx.enter_context(tc.tile_pool(name="outp", bufs=2))
        psS = ctx.enter_context(tc.tile_pool(name="psS", bufs=2, space="PSUM"))
        psA = ctx.enter_context(tc.tile_pool(name="psA", bufs=1, space="PSUM"))
        psP = ctx.enter_context(tc.tile_pool(name="psP", bufs=1, space="PSUM"))
        psV = ctx.enter_context(tc.tile_pool(name="psV", bufs=1, space="PSUM"))

        w3_sb = const.tile([128, 3, EC, D], BF16, tag="w3")
        xstage = xpool.tile([128, G, EC, 512], BF16, tag="xstage")
        enc_sb = encpool.tile([128, NTG, EC, 512], BF16, tag="enc")

        # ---- all inputs on one HWDGE queue, strict need order, 0.5MB pieces
        # so projection matmuls start on the first arrivals ----
        nc.sync.dma_start(out=w3_sb[:, 0:2], in_=wpack[:, 0:2])   # wq, wk
        if with_qbias:
            b_sb = const.tile([128, 2], F32, tag="b")
            nc.sync.dma_start(out=b_sb, in_=bpack)
        nc.sync.dma_start(out=enc_sb[:, 0, 0:4], in_=encT[0][:, 0:4])
        nc.sync.dma_start(out=enc_sb[:, 0, 4:8], in_=encT[0][:, 4:8])
        nc.sync.dma_start(out=xstage[:, 0, 0:4], in_=xT[0][:, 0:4])
        nc.sync.dma_start(out=xstage[:, 0, 4:8], in_=xT[0][:, 4:8])
        nc.sync.dma_start(out=w3_sb[:, 2:3], in_=wpack[:, 2:3])  # wv
        if with_vbias:
            bv_sb = const.tile([1, D], F32, tag="bv")
            nc.sync.dma_start(out=bv_sb,
                              in_=bv.rearrange("(o d) -> o d", o=1))
        nc.sync.dma_start(out=enc_sb[:, 1, 0:4], in_=encT[1][:, 0:4])
        nc.sync.dma_start(out=enc_sb[:, 1, 4:8], in_=encT[1][:, 4:8])
        nc.sync.dma_start(out=xstage[:, 1, 0:4], in_=xT[1][:, 0:4])
        nc.sync.dma_start(out=xstage[:, 1, 4:8], in_=xT[1][:, 4:8])
        nc.sync.dma_start(out=xstage[:, 2], in_=xT[2])
        nc.sync.dma_start(out=xstage[:, 3], in_=xT[3])

        if with_vbias:
            ones_row_f32 = const.tile([1, 128], F32, tag="ones_row_f32")
            nc.vector.memset(ones_row_f32, 1.0)
        ones_col = const.tile([128, 1], BF16, tag="ones_col")
        nc.vector.memset(ones_col, 1.0)

        # PE p-state warmup while the first DMAs land; warm_ps lives in psV
        # (rotated away only when vp(tg0) allocates, after the last filler)
        warm_sb = const.tile([128, 512], BF16, tag="warm")
        nc.vector.memset(warm_sb, 0.0)
        warm_ps = psV.tile([128, 512], F32, tag="ps_v", name="warm_ps")

        def warm(n):
            for _ in range(n):
                nc.tensor.matmul(warm_ps, lhsT=warm_sb[:, 0:128],
                                 rhs=warm_sb, start=True, stop=True,
                                 skip_group_check=True)

        warm(10)

        qT_sb = proj.tile([128, TQ], BF16, tag="qT")
        kT_sb = proj.tile([128, TKL], BF16, tag="kT")
        v_sb = proj.tile([128, C, D], BF16, tag="v")

        # ---- projections ----
        qp_box = [None]

        def qp_mm(g, c0, c1, pool):
            def step():
                if c0 == 0:
                    t = pool.tile(
                        [128, 512] if pool is psP else [128, 1024],
                        F32, tag="ps_p" if pool is psP else "ps_s",
                        name=f"qp{g}")
                    qp_box[0] = t if pool is psP else t[:, 0:512]
                for c in range(c0, c1):
                    nc.tensor.matmul(qp_box[0], lhsT=w3_sb[:, 0, c],
                                     rhs=xstage[:, g, c],
                                     start=(c == 0), stop=(c == EC - 1))
            return step

        def qp_drain(g):
            def step():
                if with_qbias:
                    nc.vector.tensor_scalar_add(qT_sb[:, ts(g, 512)],
                                                qp_box[0], b_sb[:, 0:1])
                else:
                    nc.vector.tensor_copy(qT_sb[:, ts(g, 512)], qp_box[0])
            return step

        kp_box = [None]
        vp_box = [None]

        def kp_mm(tg, c0, c1):
            def step():
                if c0 == 0:
                    kp_box[0] = psP.tile([128, 512], F32, tag="ps_p",
                                         name=f"kp{tg}")
                for c in range(c0, c1):
                    nc.tensor.matmul(kp_box[0], lhsT=w3_sb[:, 1, c],
                                     rhs=enc_sb[:, tg, c],
                                     start=(c == 0), stop=(c == EC - 1))
            return step

        def kp_drain(tg):
            def step():
                nc.vector.tensor_copy(kT_sb[:, ts(tg, 512)], kp_box[0])
            return step

        def vp_mm(tg, t):
            def step():
                if t == 0:
                    vp_box[0] = psV.tile([128, 512], F32, tag="ps_v",
                                         name=f"vp{tg}")
                if with_vbias:
                    nc.tensor.matmul(vp_box[0][:, ts(t, 128)],
                                     lhsT=ones_row_f32, rhs=bv_sb,
                                     start=True, stop=False,
                                     skip_group_check=True)
                for c in range(EC):
                    nc.tensor.matmul(vp_box[0][:, ts(t, 128)],
                                     lhsT=enc_sb[:, tg, c, ts(t, 128)],
                                     rhs=w3_sb[:, 2, c],
                                     start=(not with_vbias and c == 0),
                                     stop=(c == EC - 1),
                                     skip_group_check=True)
            return step

        def vp_drain_t(tg, t):
            # per-t drain so each key chunk's deferred PV can flush as soon
            # as its v column block exists
            def step():
                nc.vector.tensor_copy(v_sb[:, tg * 4 + t, :],
                                      vp_box[0][:, ts(t, 128)])
            return step

        # ---- attention units ----
        A12 = [None] * G
        racc = [None] * G
        deferred = {}

        def emit_pv(g, c, e12):
            if c == 0:
                A12[g] = psA.tile([128, 1024], F32, tag="ps_a",
                                  name=f"A12_{g}")
            nc.tensor.matmul(A12[g][:, 0:512], lhsT=v_sb[:, c, :],
                             rhs=e12[:, 0:512],
                             start=(c == 0), stop=(c == C - 1),
                             skip_group_check=True)
            nc.tensor.matmul(A12[g][:, 512:1024], lhsT=v_sb[:, c, :],
                             rhs=e12[:, 512:1024],
                             start=(c == 0), stop=(c == C - 1),
                             skip_group_check=True)

        def attention_unit(g, c, defer_pv=False):
            s12 = psS.tile([128, 1024], F32, tag="ps_s", name="s12")
            nc.tensor.matmul(s12[:, 0:512],
                             lhsT=kT_sb[0:64, ts(c, 128)],
                             rhs=qT_sb[0:64, ts(g, 512)],
                             start=True, stop=True, tile_position=(0, 0))
            nc.tensor.matmul(s12[:, 512:1024],
                             lhsT=kT_sb[64:128, ts(c, 128)],
                             rhs=qT_sb[64:128, ts(g, 512)],
                             start=True, stop=True, tile_position=(64, 0))
            e12 = epool.tile([128, 1024], BF16, tag="e", name=f"e_{g}_{c}")
            nc.scalar.activation(e12, s12, Exp, scale=SCALE)
            if defer_pv:
                deferred[(g, c)] = e12
            else:
                emit_pv(g, c, e12)
            if c == 0:
                racc[g] = rpool.tile([128, 1024], BF16, tag="racc",
                                     name=f"racc{g}")
                nc.vector.tensor_copy(racc[g], e12)
            else:
                nc.vector.tensor_add(racc[g], racc[g], e12)

        def pv_flush(*gcs):
            def step():
                for gc in gcs:
                    emit_pv(*gc, deferred.pop(gc))
            return step

        def r_tail(g):
            def step():
                r12p = psP.tile([65, 512], F32, tag="ps_p", name=f"r{g}")
                nc.tensor.matmul(r12p[0:1, :], lhsT=ones_col,
                                 rhs=racc[g][:, 0:512],
                                 start=True, stop=True,
                                 skip_group_check=True)
                nc.tensor.matmul(r12p[64:65, :], lhsT=ones_col,
                                 rhs=racc[g][:, 512:1024],
                                 start=True, stop=True,
                                 skip_group_check=True)
                r_sb = outp.tile([65, 512], F32, tag="r_sb", name=f"r_sb{g}")
                nc.vector.tensor_copy(r_sb, r12p)
                nc.sync.dma_start(out=rd[g, 0], in_=r_sb[0:1, :])
                nc.sync.dma_start(out=rd[g, 1], in_=r_sb[64:65, :])
            return step

        def a_drain(g):
            def step():
                out_t = outp.tile([128, 1024], BF16, tag="pv_sb",
                                  name=f"o{g}")
                if g == G - 1:
                    # ACT is idle after the last exp: split the final drain
                    # across both engines to shorten the tail chain
                    nc.scalar.copy(out_t[:, 0:512], A12[g][:, 0:512])
                    nc.vector.tensor_copy(out_t[:, 512:1024],
                                          A12[g][:, 512:1024])
                else:
                    nc.vector.tensor_copy(out_t, A12[g])
                nc.sync.dma_start(out=pvd[:, g], in_=out_t)
            return step

        # ---- schedule ----
        # prologue: kp(tg0) / qp(g0) chunk-paced behind the DMA pieces, the
        # first two vp(tg0) quarters, then the stream. All remaining
        # projection work is chopped into sub-us micro-steps attached across
        # unit slots: a briefly-blocked score matmul requeues at the BACK of
        # the PE's ready queue, so no attachment may present a multi-us burst
        # of ready work for it to fall behind. Group-0 units defer PV until
        # their v chunk drains (flush follows one slot later).
        # DMA-paced projection pieces with single-warmup fillers between them
        # so the PE never idles long enough to drop out of its p-state ramp
        kp_mm(0, 0, 4)()
        warm(1)
        kp_mm(0, 4, 8)()
        kp_drain(0)()
        warm(1)
        qp_mm(0, 0, 4, psS)()
        warm(1)
        qp_mm(0, 4, 8, psS)()
        qp_drain(0)()
        warm(1)
        vp_mm(0, 0)()
        vp_drain_t(0, 0)()
        vp_mm(0, 1)()
        vp_drain_t(0, 1)()
        attention_unit(0, 0)
        attention_unit(0, 1)

        # pre[s]: issued BEFORE unit s so writers (kp/vp drains) precede
        # their in-unit readers; qp2/qp3 in 2-matmul micro-pieces so no
        # ready-burst exceeds the per-unit PE slack
        pre = {
            2: [vp_mm(0, 2), vp_drain_t(0, 2), kp_mm(1, 0, 4)],
            3: [vp_mm(0, 3), vp_drain_t(0, 3), kp_mm(1, 4, 8), kp_drain(1)],
            4: [vp_mm(1, 0), vp_drain_t(1, 0), vp_mm(1, 1), vp_drain_t(1, 1)],
            5: [vp_mm(1, 2), vp_drain_t(1, 2), vp_mm(1, 3), vp_drain_t(1, 3),
                pv_flush((0, 4))],
            6: [qp_mm(1, 0, 4, psP), qp_mm(1, 4, 8, psP), qp_drain(1)],
            9: [qp_mm(2, 0, 2, psP)],
            10: [qp_mm(2, 2, 4, psP)],
            11: [qp_mm(2, 4, 6, psP)],
            12: [qp_mm(2, 6, 8, psP), qp_drain(2)],
            13: [r_tail(0)],
            16: [qp_mm(3, 0, 2, psP)],
            17: [qp_mm(3, 2, 4, psP)],
            18: [qp_mm(3, 4, 6, psP)],
            19: [qp_mm(3, 6, 8, psP), qp_drain(3)],
            21: [r_tail(1)],
            26: [r_tail(2)],
        }
        for s in range(2, 32):
            g, c = divmod(s, C)
            for step in pre.get(s, []):
                step()
            # unit (0,4) runs one slot before its v chunk drains
            attention_unit(g, c, defer_pv=(g == 0 and c == 4))
            if c == C - 1:
                a_drain(g)()

        r_tail(3)()

    return nc


_nc_cache = {}


def _make_bass(with_qbias: bool, with_vbias: bool):
    from concourse import bacc

    nc = bacc.Bacc("TRN2", target_bir_lowering=False, debug=False)
    _build(nc, with_qbias, with_vbias)
    nc.compile()
    return nc


def _pack_x(a):
    """[T, E] -> [T//512, 128, EC, 512] bf16 (partition-major per group)."""
    t = a.shape[0]
    at = np.ascontiguousarray(a.T.astype(NP_BF16))       # [E, T]
    at = at.reshape(EC, 128, t // 512, 512)              # [c, p, g, 512]
    return np.ascontiguousarray(at.transpose(2, 1, 0, 3))  # [g, p, c, 512]


def _pack_w(W_q, W_k, W_v):
    """3x [E, D] -> [128, 3, EC, D] bf16."""
    w = np.stack([np.asarray(w, np.float32) for w in (W_q, W_k, W_v)])
    w = w.astype(NP_BF16).reshape(3, EC, 128, D)
    return np.ascontiguousarray(w.transpose(2, 0, 1, 3))


def kernel(x, encoder_out, W_q, b_q, W_k, b_k, W_v, b_v,
           lambda_q1, lambda_k1, lambda_q2, lambda_k2, lambda_init):
    from concourse import bass_utils

    x = np.asarray(x, np.float32)
    encoder_out = np.asarray(encoder_out, np.float32)
    wpack = _pack_w(W_q, W_k, W_v)
    bpack = np.ascontiguousarray(
        np.stack([np.asarray(b_q, np.float32),
                  np.asarray(b_k, np.float32)], axis=1))  # [128, 2]
    b_v = np.asarray(b_v, np.float32)

    lam = np.float32(
        np.exp(np.float32(np.asarray(lambda_q1, np.float32)
                          @ np.asarray(lambda_k1, np.float32)))
        - np.exp(np.float32(np.asarray(lambda_q2, np.float32)
                            @ np.asarray(lambda_k2, np.float32)))
        + np.float32(np.asarray(lambda_init, np.float32))
    )

    with_qbias = bool(np.any(np.asarray(b_q, np.float32)))
    with_vbias = bool(np.any(b_v))
    key = (with_qbias, with_vbias)
    if key not in _nc_cache:
        _nc_cache[key] = _make_bass(*key)
    nc = _nc_cache[key]

    in_maps = []
    for core in range(NCORES):
        b, j = divmod(core, 2)
        xTs = _pack_x(x[b])                                   # [4,128,EC,512]
        encTs = _pack_x(encoder_out[b, j * TKL:(j + 1) * TKL])  # [2,...]
        in_maps.append({
            "xT": xTs, "encT": encTs,
            "wpack": wpack, "bpack": bpack, "bv": b_v,
        })

    res = bass_utils.run_bass_kernel_spmd(nc, in_maps,
                                          core_ids=list(range(NCORES)))
    kernel.last_result = res

    out = np.empty((B, TQ, D), np.float32)
    for b in range(B):
        p0 = np.asarray(res.results[2 * b]["pvd"], np.float32)
        p1 = np.asarray(res.results[2 * b + 1]["pvd"], np.float32)
        r0 = np.asarray(res.results[2 * b]["rd"], np.float32)
        r1 = np.asarray(res.results[2 * b + 1]["rd"], np.float32)
        A = p0 + p1              # [128, G, 1024]
        r = r0 + r1              # [G, 2, 512]
        for g in range(G):
            A1, A2 = A[:, g, 0:512], A[:, g, 512:1024]
            o = A1 / r[g, 0] - lam * (A2 / r[g, 1])   # [D, 512]
            out[b, g * 512:(g + 1) * 512, :] = o.T
    return out


# revision 30
# speedup vs baseline: 1.1463x; 1.1463x over previous
"""Differential cross-attention head on 8 Trainium2 NeuronCores.

Sharding: data-parallel over batch (4) x KEY-parallel over Tk (2) = 8 cores.
Core (b, j) computes, for ALL 2048 queries of batch b, the partial attention
sums over its 1024-key half: A1/A2 = sum_{k in half} exp(s{1,2}[k,q]) v[k]
and r1/r2 = sum_{k in half} exp(s{1,2}[k,q]). The host adds the two halves
(softmax denominators and numerators are additive over keys) and normalizes:
out = A1/r1 - lam*A2/r2, then transposes back.

Why Tk- instead of Tq-sharding: with Tq-sharding both cores of a pair
duplicate the FULL k/v projections (2048 keys); with Tk-sharding they
duplicate only the q projection, which is half the flops. Net -3.5us of
TensorE work per core, and the per-group PV accumulators drain sequentially
so PSUM fits without the a2-catch-up dance on the critical tail.

Per-core math in "transposed" orientation (host transposes the output):
  - qT = Wq^T @ xT            [D, 2048]  (4 query groups of 512)
  - kT = Wk^T @ encT          [D, 1024]  (local key half, 8 chunks of 128)
  - v  = encT^T @ Wv          [1024, D]
  - s12 = [k1.q1 | k2.q2]     [128, 1024] PSUM via PE 64-row-group tiling
  - e12 = exp(s12/8)          ScalarE, PSUM->SBUF, bf16
  - A12_g += v_chunk^T @ e12  accumulated in PSUM per query group
  - r12_g: DVE chain sum of e12 + ones-matmul partition reduce

Scheduling rules learned from traces:
  - The PE pushes a briefly-blocked instruction to the BACK of its ready
    queue, so issue order must have deps resolved by dispatch time; the
    engine then self-organizes around momentary blocks.
  - All inputs ride ONE HWDGE queue in strict need order (a single queue
    saturates all 16 DMA engines; two queues round-robin unfairly).
  - The PE p-state needs ~3us of continuous work to reach 2.4GHz; throwaway
    warmup matmuls bridge the launch-to-first-data window.
  - b_k is dropped: softmax over keys is invariant to the per-query shift
    it induces. b_q applied only when nonzero.
"""

import sys
from contextlib import ExitStack

import numpy as np

_TRN_REPO = "/opt/trn_rl_repo"
if _TRN_REPO not in sys.path:
    sys.path.insert(0, _TRN_REPO)

import ml_dtypes

import concourse.bass as bass
import concourse.tile as tile
from concourse import mybir
from concourse.bass import ds, ts

F32 = mybir.dt.float32
BF16 = mybir.dt.bfloat16

E = 1024
D = 128
B = 4
TQ = 2048
TK = 2048
NCORES = 8
EC = E // 128             # 8 contraction chunks for projections
G = TQ // 512             # 4 query groups of 512 (full batch-row of queries)
TKL = TK // 2             # 1024 local keys per core
C = TKL // 128            # 8 local key chunks
NTG = TKL // 512          # 2 local key projection groups
SCALE = 0.125             # 1/sqrt(64)

NP_BF16 = ml_dtypes.bfloat16


def _build(nc: bass.Bass, with_qbias: bool, with_vbias: bool):
    xT = nc.dram_tensor("xT", [G, 128, EC, 512], BF16,
                        kind="ExternalInput").ap()
    encT = nc.dram_tensor("encT", [NTG, 128, EC, 512], BF16,
                          kind="ExternalInput").ap()
    wpack = nc.dram_tensor("wpack", [128, 3, EC, D], BF16,
                           kind="ExternalInput").ap()
    bpack = nc.dram_tensor("bpack", [128, 2], F32, kind="ExternalInput").ap()
    bv = nc.dram_tensor("bv", [D], F32, kind="ExternalInput").ap()
    pvd = nc.dram_tensor("pvd", [128, G, 1024], BF16,
                         kind="ExternalOutput").ap()
    rd = nc.dram_tensor("rd", [G, 2, 512], F32, kind="ExternalOutput").ap()

    Exp = mybir.ActivationFunctionType.Exp

    with tile.TileContext(nc) as tc, ExitStack() as ctx:
        const = ctx.enter_context(tc.tile_pool(name="const", bufs=1))
        xpool = ctx.enter_context(tc.tile_pool(name="xpool", bufs=1))
        encpool = ctx.enter_context(tc.tile_pool(name="encpool", bufs=1))
        proj = ctx.enter_context(tc.tile_pool(name="proj", bufs=1))
        epool = ctx.enter_context(tc.tile_pool(name="epool", bufs=8))
        rpool = ctx.enter_context(tc.tile_pool(name="rpool", bufs=3))
        outp = ctx.enter_context(tc.tile_pool(name="outp", bufs=2))
        psS = ctx.enter_context(tc.tile_pool(name="psS", bufs=2, space="PSUM"))
        psA = ctx.enter_context(tc.tile_pool(name="psA", bufs=1, space="PSUM"))
        psP = ctx.enter_context(tc.tile_pool(name="psP", bufs=1, space="PSUM"))
        psV = ctx.enter_context(tc.tile_pool(name="psV", bufs=1, space="PSUM"))

        w3_sb = const.tile([128, 3, EC, D], BF16, tag="w3")
        xstage = xpool.tile([128, G, EC, 512], BF16, tag="xstage")
        enc_sb = encpool.tile([128, NTG, EC, 512], BF16, tag="enc")

        # ---- all inputs on one HWDGE queue, strict need order, 0.5MB pieces
        # so projection matmuls start on the first arrivals ----
        nc.scalar.dma_start(out=w3_sb[:, 0:2], in_=wpack[:, 0:2])   # wq, wk
        if with_qbias:
            b_sb = const.tile([128, 2], F32, tag="b")
            nc.scalar.dma_start(out=b_sb, in_=bpack)
        nc.scalar.dma_start(out=enc_sb[:, 0, 0:4], in_=encT[0][:, 0:4])
        nc.scalar.dma_start(out=enc_sb[:, 0, 4:8], in_=encT[0][:, 4:8])
        nc.scalar.dma_start(out=xstage[:, 0, 0:4], in_=xT[0][:, 0:4])
        nc.scalar.dma_start(out=xstage[:, 0, 4:8], in_=xT[0][:, 4:8])
        nc.scalar.dma_start(out=w3_sb[:, 2:3], in_=wpack[:, 2:3])  # wv
        if with_vbias:
            bv_sb = const.tile([1, D], F32, tag="bv")
            nc.scalar.dma_start(out=bv_sb,
                                in_=bv.rearrange("(o d) -> o d", o=1))
        nc.scalar.dma_start(out=enc_sb[:, 1, 0:4], in_=encT[1][:, 0:4])
        nc.scalar.dma_start(out=enc_sb[:, 1, 4:8], in_=encT[1][:, 4:8])
        nc.scalar.dma_start(out=xstage[:, 1, 0:4], in_=xT[1][:, 0:4])
        nc.scalar.dma_start(out=xstage[:, 1, 4:8], in_=xT[1][:, 4:8])
        nc.scalar.dma_start(out=xstage[:, 2], in_=xT[2])
        nc.scalar.dma_start(out=xstage[:, 3], in_=xT[3])

        if with_vbias:
            ones_row_f32 = const.tile([1, 128], F32, tag="ones_row_f32")
            nc.vector.memset(ones_row_f32, 1.0)
        ones_col = const.tile([128, 1], BF16, tag="ones_col")
        nc.vector.memset(ones_col, 1.0)

        # PE p-state warmup while the first DMAs land; warm_ps lives in psV
        # (rotated away only when vp(tg0) allocates, after the last filler)
        warm_sb = const.tile([128, 512], BF16, tag="warm")
        nc.vector.memset(warm_sb, 0.0)
        warm_ps = psV.tile([128, 512], F32, tag="ps_v", name="warm_ps")

        def warm(n):
            for _ in range(n):
                nc.tensor.matmul(warm_ps, lhsT=warm_sb[:, 0:128],
                                 rhs=warm_sb, start=True, stop=True,
                                 skip_group_check=True)

        warm(8)

        qT_sb = proj.tile([128, TQ], BF16, tag="qT")
        kT_sb = proj.tile([128, TKL], BF16, tag="kT")
        v_sb = proj.tile([128, C, D], BF16, tag="v")

        # ---- projections ----
        qp_box = [None]

        def qp_mm(g, c0, c1, pool):
            def step():
                if c0 == 0:
                    t = pool.tile(
                        [128, 512] if pool is psP else [128, 1024],
                        F32, tag="ps_p" if pool is psP else "ps_s",
                        name=f"qp{g}")
                    qp_box[0] = t if pool is psP else t[:, 0:512]
                for c in range(c0, c1):
                    nc.tensor.matmul(qp_box[0], lhsT=w3_sb[:, 0, c],
                                     rhs=xstage[:, g, c],
                                     start=(c == 0), stop=(c == EC - 1))
            return step

        def qp_drain(g):
            def step():
                if with_qbias:
                    nc.vector.tensor_scalar_add(qT_sb[:, ts(g, 512)],
                                                qp_box[0], b_sb[:, 0:1])
                else:
                    nc.vector.tensor_copy(qT_sb[:, ts(g, 512)], qp_box[0])
            return step

        kp_box = [None]
        vp_box = [None]

        def kp_mm(tg, c0, c1):
            def step():
                if c0 == 0:
                    kp_box[0] = psP.tile([128, 512], F32, tag="ps_p",
                                         name=f"kp{tg}")
                for c in range(c0, c1):
                    nc.tensor.matmul(kp_box[0], lhsT=w3_sb[:, 1, c],
                                     rhs=enc_sb[:, tg, c],
                                     start=(c == 0), stop=(c == EC - 1))
            return step

        def kp_drain(tg):
            def step():
                nc.vector.tensor_copy(kT_sb[:, ts(tg, 512)], kp_box[0])
            return step

        def vp_mm(tg, t):
            def step():
                if t == 0:
                    vp_box[0] = psV.tile([128, 512], F32, tag="ps_v",
                                         name=f"vp{tg}")
                if with_vbias:
                    nc.tensor.matmul(vp_box[0][:, ts(t, 128)],
                                     lhsT=ones_row_f32, rhs=bv_sb,
                                     start=True, stop=False,
                                     skip_group_check=True)
                for c in range(EC):
                    nc.tensor.matmul(vp_box[0][:, ts(t, 128)],
                                     lhsT=enc_sb[:, tg, c, ts(t, 128)],
                                     rhs=w3_sb[:, 2, c],
                                     start=(not with_vbias and c == 0),
                                     stop=(c == EC - 1),
                                     skip_group_check=True)
            return step

        def vp_drain_t(tg, t):
            # per-t drain so each key chunk's deferred PV can flush as soon
            # as its v column block exists
            def step():
                nc.vector.tensor_copy(v_sb[:, tg * 4 + t, :],
                                      vp_box[0][:, ts(t, 128)])
            return step

        # ---- attention units ----
        A12 = [None] * G
        racc = [None] * G
        deferred = {}

        def emit_pv(g, c, e12):
            if c == 0:
                A12[g] = psA.tile([128, 1024], F32, tag="ps_a",
                                  name=f"A12_{g}")
            nc.tensor.matmul(A12[g][:, 0:512], lhsT=v_sb[:, c, :],
                             rhs=e12[:, 0:512],
                             start=(c == 0), stop=(c == C - 1),
                             skip_group_check=True)
            nc.tensor.matmul(A12[g][:, 512:1024], lhsT=v_sb[:, c, :],
                             rhs=e12[:, 512:1024],
                             start=(c == 0), stop=(c == C - 1),
                             skip_group_check=True)

        def attention_unit(g, c, defer_pv=False):
            s12 = psS.tile([128, 1024], F32, tag="ps_s", name="s12")
            nc.tensor.matmul(s12[:, 0:512],
                             lhsT=kT_sb[0:64, ts(c, 128)],
                             rhs=qT_sb[0:64, ts(g, 512)],
                             start=True, stop=True, tile_position=(0, 0))
            nc.tensor.matmul(s12[:, 512:1024],
                             lhsT=kT_sb[64:128, ts(c, 128)],
                             rhs=qT_sb[64:128, ts(g, 512)],
                             start=True, stop=True, tile_position=(64, 0))
            e12 = epool.tile([128, 1024], BF16, tag="e", name=f"e_{g}_{c}")
            nc.scalar.activation(e12, s12, Exp, scale=SCALE)
            if defer_pv:
                deferred[(g, c)] = e12
            else:
                emit_pv(g, c, e12)
            if c == 0:
                racc[g] = rpool.tile([128, 1024], BF16, tag="racc",
                                     name=f"racc{g}")
                nc.vector.tensor_copy(racc[g], e12)
            else:
                nc.vector.tensor_add(racc[g], racc[g], e12)

        def pv_flush(*gcs):
            def step():
                for gc in gcs:
                    emit_pv(*gc, deferred.pop(gc))
            return step

        def r_tail(g):
            def step():
                r12p = psP.tile([65, 512], F32, tag="ps_p", name=f"r{g}")
                nc.tensor.matmul(r12p[0:1, :], lhsT=ones_col,
                                 rhs=racc[g][:, 0:512],
                                 start=True, stop=True,
                                 skip_group_check=True)
                nc.tensor.matmul(r12p[64:65, :], lhsT=ones_col,
                                 rhs=racc[g][:, 512:1024],
                                 start=True, stop=True,
                                 skip_group_check=True)
                r_sb = outp.tile([65, 512], F32, tag="r_sb", name=f"r_sb{g}")
                nc.vector.tensor_copy(r_sb, r12p)
                nc.sync.dma_start(out=rd[g, 0], in_=r_sb[0:1, :])
                nc.sync.dma_start(out=rd[g, 1], in_=r_sb[64:65, :])
            return step

        def a_drain(g):
            def step():
                out_t = outp.tile([128, 1024], BF16, tag="pv_sb",
                                  name=f"o{g}")
                nc.vector.tensor_copy(out_t, A12[g])
                nc.sync.dma_start(out=pvd[:, g], in_=out_t)
            return step

        # ---- schedule ----
        # prologue: kp(tg0) / qp(g0) chunk-paced behind the DMA pieces, the
        # first two vp(tg0) quarters, then the stream. All remaining
        # projection work is chopped into sub-us micro-steps attached across
        # unit slots: a briefly-blocked score matmul requeues at the BACK of
        # the PE's ready queue, so no attachment may present a multi-us burst
        # of ready work for it to fall behind. Group-0 units defer PV until
        # their v chunk drains (flush follows one slot later).
        # DMA-paced projection pieces with single-warmup fillers between them
        # so the PE never idles long enough to drop out of its p-state ramp
        kp_mm(0, 0, 4)()
        warm(1)
        kp_mm(0, 4, 8)()
        kp_drain(0)()
        warm(1)
        qp_mm(0, 0, 4, psS)()
        warm(1)
        qp_mm(0, 4, 8, psS)()
        qp_drain(0)()
        vp_mm(0, 0)()
        vp_drain_t(0, 0)()
        vp_mm(0, 1)()
        vp_drain_t(0, 1)()
        attention_unit(0, 0)
        attention_unit(0, 1)

        # pre[s]: issued BEFORE unit s so writers (kp/vp drains) precede
        # their in-unit readers; qp2/qp3 in 2-matmul micro-pieces so no
        # ready-burst exceeds the per-unit PE slack
        pre = {
            2: [vp_mm(0, 2), vp_drain_t(0, 2), kp_mm(1, 0, 4)],
            3: [vp_mm(0, 3), vp_drain_t(0, 3), kp_mm(1, 4, 8), kp_drain(1)],
            4: [vp_mm(1, 0), vp_drain_t(1, 0), vp_mm(1, 1), vp_drain_t(1, 1)],
            5: [vp_mm(1, 2), vp_drain_t(1, 2), vp_mm(1, 3), vp_drain_t(1, 3),
                pv_flush((0, 4))],
            6: [qp_mm(1, 0, 4, psP), qp_mm(1, 4, 8, psP), qp_drain(1)],
            9: [r_tail(0)],
            10: [qp_mm(2, 0, 2, psP)],
            11: [qp_mm(2, 2, 4, psP)],
            12: [qp_mm(2, 4, 6, psP)],
            13: [qp_mm(2, 6, 8, psP), qp_drain(2)],
            16: [qp_mm(3, 0, 2, psP)],
            17: [qp_mm(3, 2, 4, psP)],
            18: [qp_mm(3, 4, 6, psP)],
            19: [qp_mm(3, 6, 8, psP), qp_drain(3)],
            21: [r_tail(1)],
            26: [r_tail(2)],
        }
        for s in range(2, 32):
            g, c = divmod(s, C)
            for step in pre.get(s, []):
                step()
            # unit (0,4) runs one slot before its v chunk drains
            attention_unit(g, c, defer_pv=(g == 0 and c == 4))
            if c == C - 1:
                a_drain(g)()

        r_tail(3)()

    return nc


_nc_cache = {}


def _make_bass(with_qbias: bool, with_vbias: bool):
    from concourse import bacc

    nc = bacc.Bacc("TRN2", target_bir_lowering=False, debug=False)
    _build(nc, with_qbias, with_vbias)
    nc.compile()
    return nc


def _pack_x(a):
    """[T, E] -> [T//512, 128, EC, 512] bf16 (partition-major per group)."""
    t = a.shape[0]
    at = np.ascontiguousarray(a.T.astype(NP_BF16))       # [E, T]
    at = at.reshape(EC, 128, t // 512, 512)              # [c, p, g, 512]
    return np.ascontiguousarray(at.transpose(2, 1, 0, 3))  # [g, p, c, 512]


def _pack_w(W_q, W_k, W_v):
    """3x [E, D] -> [128, 3, EC, D] bf16."""
    w = np.stack([np.asarray(w, np.float32) for w in (W_q, W_k, W_v)])
    w = w.astype(NP_BF16).reshape(3, EC, 128, D)
    return np.ascontiguousarray(w.transpose(2, 0, 1, 3))


def kernel(x, encoder_out, W_q, b_q, W_k, b_k, W_v, b_v,
           lambda_q1, lambda_k1, lambda_q2, lambda_k2, lambda_init):
    from concourse import bass_utils

    x = np.asarray(x, np.float32)
    encoder_out = np.asarray(encoder_out, np.float32)
    wpack = _pack_w(W_q, W_k, W_v)
    bpack = np.ascontiguousarray(
        np.stack([np.asarray(b_q, np.float32),
                  np.asarray(b_k, np.float32)], axis=1))  # [128, 2]
    b_v = np.asarray(b_v, np.float32)

    lam = np.float32(
        np.exp(np.float32(np.asarray(lambda_q1, np.float32)
                          @ np.asarray(lambda_k1, np.float32)))
        - np.exp(np.float32(np.asarray(lambda_q2, np.float32)
                            @ np.asarray(lambda_k2, np.float32)))
        + np.float32(np.asarray(lambda_init, np.float32))
    )

    with_qbias = bool(np.any(np.asarray(b_q, np.float32)))
    with_vbias = bool(np.any(b_v))
    key = (with_qbias, with_vbias)
    if key not in _nc_cache:
        _nc_cache[key] = _make_bass(*key)
    nc = _nc_cache[key]

    in_maps = []
    for core in range(NCORES):
        b, j = divmod(core, 2)
        xTs = _pack_x(x[b])                                   # [4,128,EC,512]
        encTs = _pack_x(encoder_out[b, j * TKL:(j + 1) * TKL])  # [2,...]
        in_maps.append({
            "xT": xTs, "encT": encTs,
            "wpack": wpack, "bpack": bpack, "bv": b_v,
        })

    res = bass_utils.run_bass_kernel_spmd(nc, in_maps,
                                          core_ids=list(range(NCORES)))
    kernel.last_result = res

    out = np.empty((B, TQ, D), np.float32)
    for b in range(B):
        p0 = np.asarray(res.results[2 * b]["pvd"], np.float32)
        p1 = np.asarray(res.results[2 * b + 1]["pvd"], np.float32)
        r0 = np.asarray(res.results[2 * b]["rd"], np.float32)
        r1 = np.asarray(res.results[2 * b + 1]["rd"], np.float32)
        A = p0 + p1              # [128, G, 1024]
        r = r0 + r1              # [G, 2, 512]
        for g in range(G):
            A1, A2 = A[:, g, 0:512], A[:, g, 512:1024]
            o = A1 / r[g, 0] - lam * (A2 / r[g, 1])   # [D, 512]
            out[b, g * 512:(g + 1) * 512, :] = o.T
    return out
